# revision 103
# baseline (speedup 1.0000x reference)
"""GAT BasicAttentionBlock kernel for 8x Trainium2 NeuronCores.

Strategy (output-shard, v3): each core owns 1250 of the 10000 selected
output rows (index0).  Only nodes reachable from those rows matter
(~1.2k targets + ~16k unique sources per core).  Per core:

  phase A.1  tabLo tiles: stream x (bf16) for [window-grouped targets
             (1280 rows) | unique gathered-class sources]; h = relu(x@w1)
             feature-major on PE, proj|s_src = h@w2 per 128-node subtile
             (proj stored head-LAST (d,a) so later broadcasts stay off
             the packed dim); PSUM tiles copied to SBUF (Act/DVE halves)
             and DMA'd as 512B rows to the HBM table.  Per-window s_trg /
             skip / stps (one-hot expansion via Pool partition_broadcast
             + DVE compare) interleave into engine slack; the (t,j)-layout
             edge->target masks are emitted one batched DVE compare per
             window.
  gathers    all W windows' gathered-class rows dma_gather from the
             table right after phase A.1 (slot-major, 512B rows).
  phase A.2  direct tiles: one chunk per window computes proj|s_src for
             the window's mult-1 sources straight into SBUF (no table
             round-trip); the window's loop-2 stages are emitted in the
             same chunk iteration, so attention for window w overlaps
             tile compute for window w+1.
  loop 2     per window: sc = s_src[slot]+s_trg[slot]; e = exp(lrelu) as
             one fused scalar_tensor_tensor + Act exp; Wv = e (broadcast
             over the middle HD dim, 2x-packed) * proj; segment-sum via
             EC one-hot matmuls accumulated in PSUM [sum e*proj | sum e];
             out = seg/den + skip then elu(z) = max(z, exp(min(z,0))-1);
             rows stored per window via the Act HWDGE queue.
  host       the final index0 gather (table row per output row) and the
             (d,a) -> (a,d) column unpermute run in numpy after the
             device kernel; only table rows for real targets are read,
             so pad rows (denom 0 without eps) never reach the output.

No collectives: cores are fully independent.  The softmax global max
subtraction cancels in att = exp/sum(exp) and is dropped.
"""

import os
import sys

for _p in ("/opt/trn_rl_repo",):
    if os.path.isdir(_p) and _p not in sys.path:
        sys.path.insert(0, _p)

import numpy as np
import ml_dtypes

# problem constants (hardcoded per contract)
N = 50000
E = 800000
K = 10000
IN = 256
H = 128
NH = 8
HD = 16
OC = NH * HD  # 128
CORES = 8
KC = K // CORES          # 1250 output rows per core
P = 128
W = 10                   # target windows of 128 -> 1280 target slots
TP = W * P               # padded target count per core
EPS = 1e-16

BF16 = ml_dtypes.bfloat16


# ----------------------------------------------------------------------------
# host-side sharding / planning
# ----------------------------------------------------------------------------

def _wrap16(vals, reps=8):
    """int16 index layout for dma_gather: idx i at [i%16, i//16], the 16-row
    block replicated `reps` times down the partition axis."""
    L = vals.shape[0]
    assert L % 16 == 0
    w = vals.reshape(L // 16, 16).T.astype(np.int16)
    return np.tile(w, (reps, 1))


def _binpack(deg):
    """Assign targets (by degree desc) to W windows (<=128 each), balancing
    total degree.  Returns row index (w*128 + pos) per target."""
    U = len(deg)
    order = np.argsort(-deg, kind="stable")
    wdeg = np.zeros(W)
    wcnt = np.zeros(W, np.int64)
    row = np.zeros(U, np.int64)
    for u in order:
        cand = np.nonzero(wcnt < P)[0]
        wsel = cand[np.argmin(wdeg[cand])]
        row[u] = wsel * P + wcnt[wsel]
        wcnt[wsel] += 1
        wdeg[wsel] += deg[u]
    return row


def plan(x, adj0, index0):
    src_all = np.asarray(adj0[0], dtype=np.int64)
    trg_all = np.asarray(adj0[1], dtype=np.int64)
    idx0 = np.asarray(index0, dtype=np.int64)
    x = np.asarray(x, dtype=np.float32)

    pre = []
    npad_req = 512
    for c in range(CORES):
        ks = idx0[c * KC:(c + 1) * KC]
        tgt_u, inv_k = np.unique(ks, return_inverse=True)
        U_t = len(tgt_u)
        assert U_t <= TP
        lut = np.full(N, -1, np.int64)
        lut[tgt_u] = np.arange(U_t)
        tloc_all = lut[trg_all]
        sel = np.nonzero(tloc_all >= 0)[0]
        e_src = src_all[sel]
        e_tu = tloc_all[sel]
        deg = np.bincount(e_tu, minlength=U_t)
        trow = _binpack(deg)                       # tgt_u idx -> table row

        # source rows: targets keep their rows; extras sorted by edge count
        nrow = np.full(N, -1, np.int64)
        nrow[tgt_u] = trow
        is_extra = nrow[e_src] < 0
        ex_ids, ex_cnt_inv = np.unique(e_src[is_extra], return_inverse=True)
        ex_cnt = np.bincount(ex_cnt_inv)
        ex_order = np.argsort(-ex_cnt, kind="stable")
        extras = ex_ids[ex_order]
        nrow[extras] = TP + np.arange(len(extras))
        U_n = TP + len(extras)
        npad_req = max(npad_req, U_n)

        e_srow = nrow[e_src]                       # source table row per edge
        e_trow = trow[e_tu]                        # target table row per edge
        # node id per table row (for xT); pad rows -> x of node 0 (harmless)
        nodes = np.zeros(U_n, np.int64)
        nodes[trow] = tgt_u
        nodes[TP:] = extras
        pre.append((trow, inv_k, e_srow, e_trow, nodes, U_n))

    # direct class: KD slots/window of mult-1 non-target sources whose
    # proj is written straight into SBUF Ghi during phase A (no table row,
    # no gather).  Everything else is gathered from tabLo.
    KD = 8
    kg_req = 1
    packed = []
    for c in range(CORES):
        trow, inv_k, e_srow_unused, e_trow, nodes_unused, U_n = pre[c]
        # recompute from raw edge lists kept in pre
        packed.append(None)

    per_core = []
    NL_req = 512
    info = []
    for c in range(CORES):
        ks = idx0[c * KC:(c + 1) * KC]
        tgt_u, inv_k = np.unique(ks, return_inverse=True)
        lut = np.full(N, -1, np.int64)
        lut[tgt_u] = np.arange(len(tgt_u))
        tloc_all = lut[trg_all]
        sel = np.nonzero(tloc_all >= 0)[0]
        e_src = src_all[sel]
        e_tu = tloc_all[sel]
        deg = np.bincount(e_tu, minlength=len(tgt_u))
        trow = _binpack(deg)
        e_trow = trow[e_tu]
        e_win = e_trow >> 7

        is_tgt = np.zeros(N, bool)
        is_tgt[tgt_u] = True
        cnt = np.bincount(e_src, minlength=N)
        m1 = (cnt[e_src] == 1) & (~is_tgt[e_src])

        # per window: first KD*128 mult-1 edges are direct
        direct = np.zeros(len(e_src), bool)
        for w in range(W):
            idx = np.nonzero(m1 & (e_win == w))[0]
            assert len(idx) >= KD * P, (c, w, len(idx))
            direct[idx[:KD * P]] = True
        gcnt = np.bincount(e_win[~direct], minlength=W)
        kg_req = max(kg_req, int(np.ceil(gcnt.max() / P)))

        # tabLo rows: targets first, then unique gathered sources
        g_src = e_src[~direct]
        nrow = np.full(N, -1, np.int64)
        nrow[tgt_u] = trow
        extras = np.setdiff1d(np.unique(g_src), tgt_u)
        nrow[extras] = TP + np.arange(len(extras))
        NL_req = max(NL_req, TP + len(extras))
        info.append((tgt_u, inv_k, trow, e_src, e_trow, e_win, direct, nrow))

    KLO = kg_req
    KHI = KD
    EC = KLO + KHI
    cap = EC * P
    ND = W * KD * P                    # direct nodes (8960)
    NDP = ((ND + 1023) // 1024) * 1024  # padded to chunk mult (9216)
    NL = ((NL_req + 1023) // 1024) * 1024
    NPAD = NDP + NL
    B1 = NL  # tabLo tiles first; direct tiles start here

    for c in range(CORES):
        tgt_u, inv_k, trow, e_src, e_trow, e_win, direct, nrow = info[c]
        x_nodes = np.zeros(NPAD, np.int64)

        etcol = np.full((P, W * EC), -1.0, np.float32)
        esrc_g = np.zeros((W, KLO * P), np.int64)

        # direct edges: block b = w*KD + jd -> xTi position NL + b*128 + p
        for w in range(W):
            idx = np.nonzero(direct & (e_win == w))[0]
            for jd in range(KD):
                blk = idx[jd * P:(jd + 1) * P]
                b = w * KD + jd
                x_nodes[NL + b * P:NL + (b + 1) * P] = e_src[blk]
                etcol[:, w * EC + KLO + jd] = \
                    (e_trow[blk] - w * P).astype(np.float32)

        # gathered edges per window, packed into slots [0, KLO*P)
        for w in range(W):
            idx = np.nonzero((~direct) & (e_win == w))[0]
            ng = len(idx)
            esrc_g[w, :ng] = nrow[e_src[idx]]
            ec_ = np.full(KLO * P, -1.0, np.float32)
            ec_[:ng] = (e_trow[idx] - w * P).astype(np.float32)
            etcol[:, w * EC:w * EC + KLO] = \
                ec_.reshape(KLO, P).T

        # tabLo node ids at xTi positions [0, NL)
        rows_used = np.nonzero(nrow >= 0)[0]
        x_nodes[nrow[rows_used]] = rows_used

        # etrow: slot-major target row per window, replicated across all
        # partitions on the host (loaded per-window, compared on DVE)
        etrow_b = np.empty((1, W * cap), BF16)
        for w in range(W):
            etrow_b[0, w * cap:(w + 1) * cap] = \
                etcol[:, w * EC:(w + 1) * EC].T.reshape(-1).astype(BF16)

        eidx_lo = np.concatenate(
            [_wrap16(esrc_g[w]) for w in range(W)], axis=1)

        out_rows = trow[inv_k]          # host-side final gather map

        xT = np.zeros((IN, NPAD), BF16)
        xT[:, :] = x[x_nodes].T
        CW = 1024
        assert NPAD % CW == 0
        xTi = np.empty((P, 2 * NPAD), BF16)
        for i in range(NPAD // CW):
            xTi[:, 2 * i * CW:2 * i * CW + CW] = xT[0:P, i * CW:(i + 1) * CW]
            xTi[:, 2 * i * CW + CW:2 * (i + 1) * CW] = \
                xT[P:IN, i * CW:(i + 1) * CW]

        per_core.append(dict(xTi=xTi, iblob=eidx_lo,
                             etcol=np.ascontiguousarray(etcol),
                             etrow=etrow_b, out_rows=out_rows))
    return per_core, NPAD, EC, KLO, B1


def make_weights(w_in, b_in, w_proj, a_src, a_trg, w_skip, EC):
    w_in = np.asarray(w_in, np.float32)
    b_in = np.asarray(b_in, np.float32)
    w_proj = np.asarray(w_proj, np.float32)
    a_src = np.asarray(a_src, np.float32).reshape(NH, HD)
    a_trg = np.asarray(a_trg, np.float32).reshape(NH, HD)
    w_skip = np.asarray(w_skip, np.float32)

    w1T = np.ascontiguousarray(w_in.T).astype(BF16)        # [256,128]
    b1 = b_in.reshape(H, 1).astype(np.float32)
    # B_src[h, a] = sum_d w_proj[a*16+d, h] * a_src[a, d]
    wp3 = w_proj.reshape(NH, HD, H)
    B_src = np.einsum("adh,ad->ha", wp3, a_src).astype(np.float32)  # [128,8]
    B_trg = np.einsum("adh,ad->ha", wp3, a_trg).astype(BF16)
    # proj feature order is head-LAST (d, a): table col c = d*NH + a holds
    # original feature perm[c] = a*HD + d.  This keeps the attention-weight
    # multiply's broadcast off the packed last dim (DVE 2x/4x eligible).
    perm = np.array([(c % NH) * HD + c // NH for c in range(OC)])
    w2 = np.zeros((H, 144), np.float32)  # cast to bf16 below
    w2[:, :OC] = w_proj.T[:, perm]
    w2[:, OC:OC + NH] = B_src
    wskT = np.ascontiguousarray(w_skip.T[:, perm]).astype(BF16)  # [128,128]
    # miota[p, t*EC + j] = t : compare target for the (t, j)-layout masks
    miota = np.repeat(np.arange(P).astype(BF16), EC)[None, :].repeat(P, axis=0)
    iota_c = np.arange(P, dtype=np.float32).reshape(P, 1)
    bfblob = np.concatenate(
        [np.ascontiguousarray(w1T[0:P]), np.ascontiguousarray(w1T[P:IN]),
         w2.astype(BF16), wskT, B_trg], axis=1)  # [128, 536]
    return dict(bfblob=bfblob, miota=np.ascontiguousarray(miota), b1=b1,
                iota_c=iota_c, invperm=np.argsort(perm))


# ----------------------------------------------------------------------------
# bass kernel
# ----------------------------------------------------------------------------

_BUILD_CACHE = {}


def build(NPAD, EC, KLO, B1):
    key = (NPAD, EC, KLO, B1)
    if key in _BUILD_CACHE:
        return _BUILD_CACHE[key]

    import concourse.bacc as bacc
    import concourse.mybir as mybir
    import concourse.tile as tile

    dt = mybir.dt
    F32 = dt.float32
    F32R = dt.float32r
    I16 = dt.int16
    BF = dt.bfloat16
    AF = mybir.ActivationFunctionType
    OP = mybir.AluOpType

    NT = NPAD // 512
    cap = EC * P
    KHI = EC - KLO

    nc = bacc.Bacc("TRN2", target_bir_lowering=False,
                   num_swdge_queues=4)

    with tile.TileContext(nc) as tc:
        with tc.tile_pool(name="dram", bufs=1, space="DRAM") as dram:
            def din(name, shape, dtp):
                return dram.tile(shape, dtp, kind="ExternalInput", name=name,
                                 uniquify=False)

            NBF = H + H + 144 + OC + NH
            NI16 = W * KLO * 8
            xTi = din("xTi", [P, 2 * NPAD], BF)
            bfblob = din("bfblob", [P, NBF], BF)
            fblob = din("fblob", [P, 2], F32)
            iblob = din("iblob", [P, NI16], I16)
            etrow = din("etrow", [1, W * cap], BF)
            etcb = din("etcb", [P, W * EC], BF)
            miot = din("miot", [P, EC * P], BF)

            tabLo = dram.tile([NPAD - B1, 256], BF, kind="Internal",
                              name="tabLo", uniquify=False)
            out = dram.tile([TP, OC], BF, kind="ExternalOutput", name="out",
                            uniquify=False)

        with tc.tile_pool(name="pers", bufs=1) as pers:
            bfb = pers.tile([P, NBF], BF)
            fb = pers.tile([P, 2], F32)
            ib = pers.tile([P, NI16], I16)
            ecb = pers.tile([P, W * EC], BF)
            etws = pers.tile([1, W * cap], BF)
            miosb = pers.tile([P, EC * P], BF)
            hfmt = pers.tile([H, TP], BF)         # targets' h, feature-major
            strg = pers.tile([P, W * NH], BF)     # per-window s_trg  [t, 8]
            skips = pers.tile([P, W, OC], BF)     # per-window skip   [t, oc]
            st_sb = pers.tile([P, W, EC, NH], BF)   # s_trg per edge slot

            Ghi = pers.tile([P, W, EC - KLO, 136], BF)  # direct-class rows

            nc.sync.dma_start(etws[:], etrow[:])
            nc.sync.dma_start(fb[:], fblob[:])
            nc.sync.dma_start(bfb[:], bfblob[:])

            w1a = bfb[:, 0:H]
            w1b = bfb[:, H:2 * H]
            w2s = bfb[:, 2 * H:2 * H + 144]
            wsks = bfb[:, 2 * H + 144:2 * H + 144 + OC]
            btrgs = bfb[:, 2 * H + 144 + OC:2 * H + 144 + OC + NH]
            miotas = miosb[:, :]
            b1s = fb[:, 0:1]
            iotac = fb[:, 1:2]
            eloidx = ib[:, 0:W * KLO * 8]

            CH = 2  # 512-node tiles per xT load chunk
            LT = B1 // 512  # tabLo tiles
            with tc.tile_pool(name="pa", bufs=2) as pa, \
                 tc.tile_pool(name="pax", bufs=4) as pax, \
                 tc.tile_pool(name="pbc", bufs=2) as pbc, \
                 tc.tile_pool(name="pmw", bufs=10) as pmw, \
                 tc.tile_pool(name="pmtw", bufs=1) as pmtw, \
                 tc.tile_pool(name="pghi", bufs=8) as pghi, \
                 tc.tile_pool(name="pe2", bufs=2) as pe2, \
                 tc.tile_pool(name="psa", bufs=2, space="PSUM") as psa, \
                 tc.tile_pool(name="psb", bufs=2, space="PSUM") as psb, \
                 tc.tile_pool(name="psc", bufs=1, space="PSUM") as psc, \
                 tc.tile_pool(name="psd", bufs=1, space="PSUM") as psd, \
                 tc.tile_pool(name="pse", bufs=2, space="PSUM") as pse:

                # deferred emissions interleaved into phase A slack
                mtws = {}

                def emit_mtw(w):
                    # partition-broadcast of per-slot target cols (Pool),
                    # emitted just-in-time to keep the Pool queue in step
                    pbcw = pbc.tile([P, cap], BF, tag="pbcw")
                    nc.gpsimd.partition_broadcast(
                        pbcw[:], etws[0:1, w * cap:(w + 1) * cap])
                    Mtw = pmtw.tile([P, cap], BF, tag="Mtw")
                    nc.vector.tensor_scalar(Mtw[:], pbcw[:], iotac[:], None,
                                            OP.is_equal)
                    mtws[w] = Mtw

                def emit_loop1(w):
                    # s_trg / skip for the window targets
                    stp = psd.tile([P, OC], F32, tag="misc")
                    nc.tensor.matmul(stp[:, 0:NH],
                                     lhsT=hfmt[:, w * P:(w + 1) * P],
                                     rhs=btrgs[:], start=True, stop=True)
                    nc.vector.tensor_copy(strg[:, w * NH:(w + 1) * NH],
                                            stp[:, 0:NH])
                    skp = psd.tile([P, OC], F32, tag="misc")
                    nc.tensor.matmul(skp[:], lhsT=hfmt[:, w * P:(w + 1) * P],
                                     rhs=wsks[:], start=True, stop=True)
                    nc.scalar.activation(skips[:, w], skp[:], AF.Copy)
                    # s_trg edge-slot expansion via the col-major one-hot
                    Mtw = mtws.pop(w)
                    stps = psc.tile([P, EC, NH], F32, tag="stps")
                    for j in range(EC):
                        nc.tensor.matmul(
                            stps[:, j, :], lhsT=Mtw[:, j * P:(j + 1) * P],
                            rhs=strg[:, w * NH:(w + 1) * NH],
                            start=True, stop=True)
                    nc.vector.tensor_copy(st_sb[:, w], stps[:])

                mws = {}

                def emit_mw(w):
                    # (t, j) layout: one packed compare per window
                    Mww = pmw.tile([P, cap], BF, tag="mww")
                    nc.vector.tensor_tensor(
                        Mww[:].rearrange("p (t j) -> p t j", j=EC),
                        miotas[:].rearrange("p (t j) -> p t j", j=EC),
                        ecb[:, w * EC:(w + 1) * EC].unsqueeze(1).broadcast_to(
                            [P, P, EC]),
                        OP.is_equal)
                    mws[w] = Mww

                def emit_interleave(pi):
                    # mtw at odd positions, loop1 two behind at even ones
                    if 1 <= pi <= 2 * W and pi % 2 == 1:
                        emit_mtw((pi - 1) // 2)
                    if 2 <= pi <= 2 * W + 1 and pi % 2 == 0:
                        emit_loop1((pi - 2) // 2)
                    if 4 <= pi <= 3 + W:
                        emit_mw(pi - 4)

                def do_tile(t, o, wdc, xc, stg):
                    hps = psa.tile([P, 512], F32, tag="hps")
                    nc.tensor.matmul(hps[:], lhsT=w1a[:],
                                     rhs=xc[:, o:o + 512],
                                     start=True, stop=False)
                    nc.tensor.matmul(hps[:], lhsT=w1b[:],
                                     rhs=xc[:, wdc + o:wdc + o + 512],
                                     start=False, stop=True)
                    hsb = pa.tile([P, 512], BF, tag="hsb")
                    nc.scalar.activation(hsb[:], hps[:], AF.Relu,
                                         bias=b1s[:])
                    if t * 512 < TP:
                        w0 = t * 512
                        w1_ = min(TP, w0 + 512)
                        nc.scalar.activation(hfmt[:, w0:w1_],
                                             hps[:, 0:(w1_ - w0)], AF.Relu,
                                             bias=b1s[:])
                    for half in range(2):
                        p2 = psb.tile([P, 2, 144], F32, tag="p2")
                        for jj in range(2):
                            j = half * 2 + jj
                            nc.tensor.matmul(
                                p2[:, jj, :],
                                lhsT=hsb[:, j * P:(j + 1) * P],
                                rhs=w2s[:], start=True, stop=True)
                        if t >= LT:
                            # direct class: copy straight into Ghi; the
                            # host packed these nodes in edge-slot order
                            b0 = (t - LT) * 4 + half * 2
                            w_, jd = b0 // KHI, b0 % KHI
                            dst = Ghi[:, w_, jd:jd + 2, 0:OC + NH]
                            s2 = p2[:, :, 0:OC + NH]
                            if half == 0:
                                nc.scalar.activation(dst, s2, AF.Copy)
                            else:
                                nc.vector.tensor_copy(dst, s2)
                        else:
                            sgh = stg[:, t % CH, half * 2:half * 2 + 2, :]
                            if half == 0:
                                nc.scalar.activation(sgh[:, :, 0:OC + NH],
                                                     p2[:, :, 0:OC + NH],
                                                     AF.Copy)
                            else:
                                nc.vector.tensor_copy(sgh[:, :, 0:OC + NH],
                                                      p2[:, :, 0:OC + NH])

                # ---------------- loop 2: per-window edge pipeline ----------
                # software-pipelined window stages: each engine's
                # in-order queue interleaves adjacent windows
                glos = {}
                st1 = {}   # w -> (sc-dependent) emax tile
                st2 = {}   # w -> Wv tile
                st3 = {}   # w -> segp psum tile

                def stage1(w):
                    G = glos[w]
                    sc = pe2.tile([P, EC, NH], F32, tag="sc")
                    nc.vector.tensor_tensor(sc[:, 0:KLO], st_sb[:, w, 0:KLO],
                                            G[:, :, OC:OC + NH], OP.add)
                    nc.vector.tensor_tensor(sc[:, KLO:EC],
                                            st_sb[:, w, KLO:EC],
                                            Ghi[:, w, :, OC:OC + NH], OP.add)
                    # exp(lrelu(s)): lrelu = (s*0.2) max s in one fused op
                    e2 = pe2.tile([P, EC, NH], BF, tag="e2")
                    nc.vector.scalar_tensor_tensor(e2[:], sc[:], 0.2, sc[:],
                                                   OP.mult, OP.max)
                    e1 = pe2.tile([P, EC, NH], BF, tag="e1")
                    nc.scalar.activation(e1[:], e2[:], AF.Exp)
                    st1[w] = e1

                def stage2(w):
                    # proj cols are (d, a): e broadcasts over the MIDDLE HD
                    # dim, last dim stays packed -> DVE 2x/4x mode
                    G = glos[w]
                    emax = st1.pop(w)
                    Wv = pe2.tile([P, EC, 136], BF, tag="Wv")
                    nc.vector.tensor_copy(Wv[:, :, OC:OC + NH], emax[:])
                    nc.vector.tensor_tensor(
                        Wv[:, 0:KLO, 0:OC].rearrange(
                            "p j (d a) -> p j d a", a=NH),
                        G[:, :, 0:OC].rearrange("p j (d a) -> p j d a", a=NH),
                        emax[:, 0:KLO].unsqueeze(2).broadcast_to(
                            [P, KLO, HD, NH]),
                        OP.mult)
                    nc.vector.tensor_tensor(
                        Wv[:, KLO:EC, 0:OC].rearrange(
                            "p j (d a) -> p j d a", a=NH),
                        Ghi[:, w, :, 0:OC].rearrange(
                            "p j (d a) -> p j d a", a=NH),
                        emax[:, KLO:EC].unsqueeze(2).broadcast_to(
                            [P, KHI, HD, NH]),
                        OP.mult)
                    st2[w] = Wv

                def stage3(w):
                    Wv = st2.pop(w)
                    mwin = mws.pop(w)[:].rearrange(
                        "p (t j) -> p j t", j=EC)
                    segp = pse.tile([P, 136], F32, tag="segp")
                    for j in range(EC):
                        nc.tensor.matmul(segp[:], lhsT=mwin[:, j],
                                         rhs=Wv[:, j, :], start=(j == 0),
                                         stop=(j == EC - 1))
                    st3[w] = segp

                def finalize(w):
                    segp = st3.pop(w)
                    # eps dropped: every real target has >=1 edge, and
                    # pad rows (denom 0 -> inf/nan) are never gathered
                    rec = pe2.tile([P, NH], F32, tag="rec")
                    nc.vector.reciprocal(rec[:], segp[:, OC:OC + NH])
                    z = pe2.tile([P, OC], F32, tag="z")
                    recb = rec[:].unsqueeze(1).broadcast_to([P, HD, NH])
                    nc.vector.tensor_tensor(
                        z[:].rearrange("p (d a) -> p d a", a=NH),
                        segp[:, 0:OC].rearrange("p (d a) -> p d a", a=NH),
                        recb, OP.mult)
                    nc.gpsimd.tensor_add(z[:], z[:], skips[:, w])
                    # elu(z) = max(z, exp(min(z,0)) - 1)
                    bm = pe2.tile([P, OC], BF, tag="bm")
                    nc.gpsimd.tensor_scalar(bm[:], z[:], 0.0, None, OP.min)
                    eb = pe2.tile([P, OC], BF, tag="eb")
                    nc.scalar.activation(eb[:], bm[:], AF.Exp)
                    am = pe2.tile([P, OC], BF, tag="am")
                    nc.vector.scalar_tensor_tensor(am[:], eb[:], -1.0, z[:],
                                                   OP.add, OP.max)
                    nc.scalar.dma_start(out[w * P:(w + 1) * P, :], am[:])

                # ------- phase A part 1: tabLo tiles (targets + gathered) ---
                wdc = CH * 512
                for ci in range(LT // CH):
                    t0 = ci * CH
                    xc = pax.tile([P, 2 * wdc], BF, tag="xc")
                    nc.sync.dma_start(xc[:], xTi[:, 2 * ci * wdc:
                                                 2 * (ci + 1) * wdc])
                    stg = pa.tile([P, 2, 4, 256], BF, tag="stg")
                    for t in range(t0, t0 + CH):
                        do_tile(t, (t - t0) * 512, wdc, xc, stg)
                    rr = t0 * 512
                    nc.sync.dma_start(
                        tabLo[rr:rr + CH * 512, :].rearrange(
                            "(i j p) f -> p i j f", p=P, i=CH), stg[:])
                    if ci == 0:
                        nc.sync.dma_start(ecb[:], etcb[:])
                        nc.sync.dma_start(miosb[:], miot[:])
                    elif ci == 1:
                        nc.sync.dma_start(ib[:], iblob[:])
                    for pi in (t0, t0 + 1):
                        emit_interleave(pi)

                # ------- gathers: issue all as soon as tabLo is complete ----
                for w in range(W):
                    G = pghi.tile([P, KLO, 256], BF, tag="G")
                    nc.gpsimd.dma_gather(
                        G[:], tabLo[:],
                        eloidx[:, w * KLO * 8:(w + 1) * KLO * 8],
                        KLO * P, KLO * P, 256, single_packet=False,
                        queue_num=1 + w % 3)
                    glos[w] = G

                # ------- phase A part 2: direct tiles + loop-2 pipeline -----
                # chunk cd completes Ghi window cd; loop-2 stages trail it
                DC = (NT - LT) // CH
                assert DC == W and KHI % 2 == 0
                for cd in range(DC):
                    ci = LT // CH + cd
                    t0 = ci * CH
                    xc = pax.tile([P, 2 * wdc], BF, tag="xc")
                    nc.sync.dma_start(xc[:], xTi[:, 2 * ci * wdc:
                                                 2 * (ci + 1) * wdc])
                    for t in range(t0, t0 + CH):
                        do_tile(t, (t - t0) * 512, wdc, xc, None)
                    for pi in (t0, t0 + 1):
                        emit_interleave(pi)
                    stage1(cd)
                    stage2(cd)
                    stage3(cd)
                    if cd >= 1:
                        finalize(cd - 1)
                finalize(W - 1)

    nc.compile()
    _BUILD_CACHE[key] = nc
    return nc


# ----------------------------------------------------------------------------
# entry point
# ----------------------------------------------------------------------------

def kernel(x, adj0, index0, w_in, b_in, w_proj, a_src, a_trg, w_skip):
    from concourse.bass_utils import run_bass_kernel_spmd

    per_core, NPAD, EC, KLO, B1 = plan(x, adj0, index0)
    wts = make_weights(w_in, b_in, w_proj, a_src, a_trg, w_skip, EC)
    nc = build(NPAD, EC, KLO, B1)

    fblob = np.concatenate(
        [wts["b1"], wts["iota_c"]], axis=1).astype(np.float32)
    in_maps = []
    for c in range(CORES):
        pc = per_core[c]
        in_maps.append(dict(bfblob=wts["bfblob"], fblob=fblob,
                            xTi=pc["xTi"], iblob=pc["iblob"],
                            etrow=pc["etrow"], miot=wts["miota"],
                            etcb=pc["etcol"].astype(BF16)))

    res = run_bass_kernel_spmd(nc, in_maps, core_ids=list(range(CORES)))
    inv = wts["invperm"]
    outs = [r["out"][per_core[c]["out_rows"]][:, inv]
            for c, r in enumerate(res.results)]
    return np.concatenate(outs, axis=0).astype(np.float32)

